# revision 4
# baseline (speedup 1.0000x reference)
"""Trainium2 Bass kernel for a 1-layer transformer encoder (N=10, S=1024, H=768, A=12, F=3072).

Sharding: 8 cores, SPMD, no collectives. Core k computes:
  - primary:   full batch element n=k           (1024 query rows)
  - secondary: rows [256*b, 256*(b+1)) of n=8+k//4, b=k%4   (256 query rows)
Every core runs an identical program over 1280 query rows; K/V for the
secondary batch element are recomputed per core (small duplicated cost, no
cross-core communication).

Matmuls run in bf16 (fp32 PSUM accumulation); softmax/LayerNorm statistics in fp32.
"""
import numpy as np
from contextlib import ExitStack

import ml_dtypes

N, S, H, A, F = 10, 1024, 768, 12, 3072
DH = H // A  # 64
EPS = 1e-5
SQ = 1280          # query rows per core
N_CORES = 8
HC = H // 128      # 6  h-chunks
FC = F // 128      # 24 f-chunks
PAIRS = A // 2     # 6  head pairs

_RUNNER = None


def _build_program():
    import concourse.mybir as mybir
    import concourse.tile as tile
    from concourse import bacc
    from concourse.masks import make_identity

    BF = mybir.dt.bfloat16
    F32 = mybir.dt.float32
    AF = mybir.ActivationFunctionType
    OP = mybir.AluOpType

    nc = bacc.Bacc("TRN2", target_bir_lowering=False, debug=False, num_devices=N_CORES)

    d = {}
    def din(name, shape, dt):
        d[name] = nc.dram_tensor(name, shape, dt, kind="ExternalInput").ap()

    din("xt_q", [H, SQ], BF)      # X^T of the 1280 query rows (cols 0:1024 = primary X^T)
    din("xt_s", [H, S], BF)       # X^T of the secondary batch element (K/V source)
    din("x_res", [SQ, H], F32)    # X rows for the first residual
    din("wq", [H, H], BF)         # [h, a*64+d]
    din("wk", [H, H], BF)
    din("wv", [H, H], BF)
    din("wo", [H, H], BF)         # [a*64+d, h']
    din("w1", [H, F], BF)
    din("w2", [F, H], BF)
    din("b1c", [128, FC], F32)    # col j = b1[128j:128(j+1)]
    din("b2r", [128, H], F32)     # b2 replicated over partitions
    din("gr", [128, H], F32)      # gamma replicated
    din("br", [128, H], F32)      # beta replicated
    out_d = nc.dram_tensor("out", [SQ, H], F32, kind="ExternalOutput").ap()
    x1_scratch = nc.dram_tensor("x1_scratch", [SQ, H], F32).ap()  # internal DRAM

    S_BLOCKS = [(0, 512), (512, 512), (1024, 256)]  # query-dim blocking

    with tile.TileContext(nc) as tc, ExitStack() as ctx:
        glob = ctx.enter_context(tc.tile_pool(name="glob", bufs=1))
        x1t = glob.tile([128, HC, SQ], BF)        # X1^T for FFN1
        ident = glob.tile([128, 128], BF)
        make_identity(nc, ident)
        eps_t = glob.tile([128, 1], F32)
        nc.vector.memset(eps_t, EPS)
        gr_t = glob.tile([128, H], F32)
        nc.sync.dma_start(out=gr_t, in_=d["gr"])
        br_t = glob.tile([128, H], F32)
        nc.sync.dma_start(out=br_t, in_=d["br"])
        b2r_t = glob.tile([128, H], F32)
        nc.sync.dma_start(out=b2r_t, in_=d["b2r"])
        b1c_t = glob.tile([128, FC], F32)
        nc.sync.dma_start(out=b1c_t, in_=d["b1c"])

        def ln_inplace(pool, x):
            """LayerNorm x (f32 [128, 768]) in place: (x-mu)*rsqrt(var+eps)*gamma+beta."""
            stats = pool.tile([128, 2, 6], F32)
            nc.vector.bn_stats(stats[:, 0, :], x[:, 0:384])
            nc.vector.bn_stats(stats[:, 1, :], x[:, 384:768])
            mv = pool.tile([128, 2], F32)
            nc.vector.bn_aggr(mv, stats)
            std = pool.tile([128, 1], F32)
            nc.scalar.activation(std, mv[:, 1:2], AF.Sqrt, bias=eps_t)
            rstd = pool.tile([128, 1], F32)
            nc.vector.reciprocal(rstd, std)
            nc.vector.tensor_scalar(x, x, mv[:, 0:1], rstd, OP.subtract, OP.mult)
            nc.vector.tensor_mul(x, x, gr_t)
            nc.vector.tensor_add(x, x, br_t)

        with ExitStack() as actx:  # ---- attention phase (projections .. O/LN1) ----
            apool = actx.enter_context(tc.tile_pool(name="apool", bufs=1))
            qt = apool.tile([128, PAIRS, SQ], BF)     # Q^T/8, head-pair-major rows
            zt = apool.tile([128, PAIRS, SQ], BF)     # Z^T
            kt = [apool.tile([128, PAIRS, S], BF, name=f"kt{s}", tag=f"kt{s}") for s in range(2)]
            vt = [apool.tile([128, 8, H], BF, name=f"vt{s}", tag=f"vt{s}") for s in range(2)]  # V normal, t-chunked

            with ExitStack() as pctx:  # -- projections --
                ppool = pctx.enter_context(tc.tile_pool(name="ppool", bufs=1))
                xtq_t = ppool.tile([128, HC, SQ], BF)
                nc.sync.dma_start(out=xtq_t, in_=d["xt_q"].rearrange("(c p) s -> p c s", p=128))
                xts_t = ppool.tile([128, HC, S], BF)
                nc.sync.dma_start(out=xts_t, in_=d["xt_s"].rearrange("(c p) s -> p c s", p=128))
                wq_t = ppool.tile([128, HC, H], BF)
                nc.sync.dma_start(out=wq_t, in_=d["wq"].rearrange("(c p) n -> p c n", p=128))
                wk_t = ppool.tile([128, HC, H], BF)
                nc.sync.dma_start(out=wk_t, in_=d["wk"].rearrange("(c p) n -> p c n", p=128))
                wv_t = ppool.tile([128, HC, H], BF)
                nc.sync.dma_start(out=wv_t, in_=d["wv"].rearrange("(c p) n -> p c n", p=128))

                ps512 = pctx.enter_context(tc.tile_pool(name="ps512", bufs=3, space="PSUM"))
                ps768 = pctx.enter_context(tc.tile_pool(name="ps768", bufs=2, space="PSUM"))

                # Q^T (scaled by 1/8): out [d-pair 128, s] = wq_pair^T @ X^T
                for pair in range(PAIRS):
                    for s0, sl in S_BLOCKS:
                        ps = ps512.tile([128, 512], F32)
                        for kc in range(HC):
                            nc.tensor.matmul(
                                ps[:, :sl],
                                wq_t[:, kc, pair * 128:(pair + 1) * 128],
                                xtq_t[:, kc, s0:s0 + sl],
                                start=(kc == 0), stop=(kc == HC - 1))
                        nc.scalar.activation(qt[:, pair, s0:s0 + sl], ps[:, :sl],
                                             AF.Copy, scale=0.125)

                for st in range(2):  # KV sets: 0 = primary, 1 = secondary
                    src = xtq_t if st == 0 else xts_t
                    # K^T
                    for pair in range(PAIRS):
                        for t0 in (0, 512):
                            ps = ps512.tile([128, 512], F32)
                            for kc in range(HC):
                                nc.tensor.matmul(
                                    ps,
                                    wk_t[:, kc, pair * 128:(pair + 1) * 128],
                                    src[:, kc, t0:t0 + 512],
                                    start=(kc == 0), stop=(kc == HC - 1))
                            nc.vector.tensor_copy(kt[st][:, pair, t0:t0 + 512], ps)
                    # V (normal layout): out [t-chunk 128, a*64+d]
                    for t8 in range(8):
                        ps = ps768.tile([128, H], F32)
                        for kc in range(HC):
                            nc.tensor.matmul(ps[:, 0:512], src[:, kc, t8 * 128:(t8 + 1) * 128],
                                             wv_t[:, kc, 0:512],
                                             start=(kc == 0), stop=(kc == HC - 1))
                            nc.tensor.matmul(ps[:, 512:768], src[:, kc, t8 * 128:(t8 + 1) * 128],
                                             wv_t[:, kc, 512:768],
                                             start=(kc == 0), stop=(kc == HC - 1))
                        nc.vector.tensor_copy(vt[st][:, t8, :], ps)

            with ExitStack() as attx:  # -- attention core --
                at_f = attx.enter_context(tc.tile_pool(name="at_f", bufs=3))
                at_b = attx.enter_context(tc.tile_pool(name="at_b", bufs=3))
                dens = attx.enter_context(tc.tile_pool(name="dens", bufs=6))
                ps_sc = attx.enter_context(tc.tile_pool(name="ps_sc", bufs=2, space="PSUM"))
                ps_at = attx.enter_context(tc.tile_pool(name="ps_at", bufs=1, space="PSUM"))
                ps_z = attx.enter_context(tc.tile_pool(name="ps_z", bufs=2, space="PSUM"))

                for a in range(A):
                    pair, r0 = a // 2, (a % 2) * 64
                    for sc in range(10):
                        st = 0 if sc < 8 else 1
                        ps = ps_sc.tile([128, 1024], F32)
                        q_ap = qt[r0:r0 + 64, pair, sc * 128:(sc + 1) * 128]
                        nc.tensor.matmul(ps[:, 0:512], q_ap, kt[st][r0:r0 + 64, pair, 0:512],
                                         start=True, stop=True)
                        nc.tensor.matmul(ps[:, 512:1024], q_ap, kt[st][r0:r0 + 64, pair, 512:1024],
                                         start=True, stop=True)
                        attn = at_f.tile([128, 1024], F32)
                        den = dens.tile([128, 2], F32)
                        nc.scalar.activation(attn[:, 0:512], ps[:, 0:512], AF.Exp,
                                             accum_out=den[:, 0:1])
                        nc.scalar.activation(attn[:, 512:1024], ps[:, 512:1024], AF.Exp,
                                             accum_out=den[:, 1:2])
                        densum = dens.tile([128, 1], F32)
                        nc.vector.tensor_add(densum, den[:, 0:1], den[:, 1:2])
                        recip = dens.tile([128, 1], F32)
                        nc.vector.reciprocal(recip, densum)
                        attn_b = at_b.tile([128, 1024], BF)
                        nc.vector.tensor_scalar_mul(attn_b, attn, recip)
                        # transpose attn -> attnT via PE, then Z^T = V^T-chunks @ attnT
                        ps_t = ps_at.tile([128, 1024], F32)
                        for t8 in range(8):
                            nc.tensor.matmul(ps_t[:, t8 * 128:(t8 + 1) * 128],
                                             attn_b[:, t8 * 128:(t8 + 1) * 128], ident,
                                             start=True, stop=True)
                        at_sb = at_b.tile([128, 1024], BF, tag="at_sb")
                        nc.vector.tensor_copy(at_sb[:, 0:512], ps_t[:, 0:512])
                        nc.vector.tensor_copy(at_sb[:, 512:1024], ps_t[:, 512:1024])
                        ps_zt = ps_z.tile([64, 128], F32)
                        for t8 in range(8):
                            nc.tensor.matmul(ps_zt, vt[st][:, t8, a * 64:(a + 1) * 64],
                                             at_sb[:, t8 * 128:(t8 + 1) * 128],
                                             start=(t8 == 0), stop=(t8 == 7))
                        nc.vector.tensor_copy(zt[r0:r0 + 64, pair, sc * 128:(sc + 1) * 128], ps_zt)

            with ExitStack() as octx:  # -- O projection + residual + LN1 --
                opool = octx.enter_context(tc.tile_pool(name="opool", bufs=1))
                wo_t = opool.tile([128, HC, H], BF)
                nc.sync.dma_start(out=wo_t, in_=d["wo"].rearrange("(c p) n -> p c n", p=128))
                otmp = octx.enter_context(tc.tile_pool(name="otmp", bufs=3))
                ostat = octx.enter_context(tc.tile_pool(name="ostat", bufs=6))
                ps_o = octx.enter_context(tc.tile_pool(name="ps_o", bufs=2, space="PSUM"))
                ps_tr = octx.enter_context(tc.tile_pool(name="ps_tr", bufs=2, space="PSUM"))

                for sc in range(10):
                    ps = ps_o.tile([128, H], F32)
                    for kc in range(HC):
                        z_ap = zt[:, kc, sc * 128:(sc + 1) * 128]
                        nc.tensor.matmul(ps[:, 0:512], z_ap, wo_t[:, kc, 0:512],
                                         start=(kc == 0), stop=(kc == HC - 1))
                        nc.tensor.matmul(ps[:, 512:768], z_ap, wo_t[:, kc, 512:768],
                                         start=(kc == 0), stop=(kc == HC - 1))
                    xr = otmp.tile([128, H], F32, tag="xr")
                    nc.sync.dma_start(out=xr, in_=d["x_res"][sc * 128:(sc + 1) * 128, :])
                    x1c = otmp.tile([128, H], F32, tag="x1c")
                    nc.vector.tensor_add(x1c, ps, xr)
                    ln_inplace(ostat, x1c)
                    nc.sync.dma_start(out=x1_scratch[sc * 128:(sc + 1) * 128, :], in_=x1c)
                    x1b = otmp.tile([128, H], BF, tag="x1b")
                    nc.gpsimd.tensor_copy(x1b, x1c)
                    for hc2 in range(HC):
                        ps_x = ps_tr.tile([128, 128], F32)
                        nc.tensor.matmul(ps_x, x1b[:, hc2 * 128:(hc2 + 1) * 128], ident,
                                         start=True, stop=True)
                        nc.vector.tensor_copy(x1t[:, hc2, sc * 128:(sc + 1) * 128], ps_x)

        with ExitStack() as fctx:  # ---- FFN phase ----
            fpool = fctx.enter_context(tc.tile_pool(name="fpool", bufs=1))
            w1_t = fpool.tile([128, HC, F], BF)
            nc.sync.dma_start(out=w1_t, in_=d["w1"].rearrange("(c p) n -> p c n", p=128))
            w2_t = fpool.tile([128, FC, H], BF)
            nc.sync.dma_start(out=w2_t, in_=d["w2"].rearrange("(c p) n -> p c n", p=128))
            hidp = fctx.enter_context(tc.tile_pool(name="hidp", bufs=2))
            ftmp = fctx.enter_context(tc.tile_pool(name="ftmp", bufs=3))
            fstat = fctx.enter_context(tc.tile_pool(name="fstat", bufs=6))
            ps_h = fctx.enter_context(tc.tile_pool(name="ps_h", bufs=3, space="PSUM"))
            ps_f = fctx.enter_context(tc.tile_pool(name="ps_f", bufs=2, space="PSUM"))

            for s0, sl in S_BLOCKS:
                hid = hidp.tile([128, FC, 512], BF)
                for fc in range(FC):
                    ps = ps_h.tile([128, 512], F32)
                    for kc in range(HC):
                        nc.tensor.matmul(ps[:, :sl], w1_t[:, kc, fc * 128:(fc + 1) * 128],
                                         x1t[:, kc, s0:s0 + sl],
                                         start=(kc == 0), stop=(kc == HC - 1))
                    nc.scalar.activation(hid[:, fc, :sl], ps[:, :sl], AF.Gelu,
                                         bias=b1c_t[:, fc:fc + 1])
                for scl in range(sl // 128):
                    sc = s0 // 128 + scl
                    ps = ps_f.tile([128, H], F32)
                    for fc in range(FC):
                        h_ap = hid[:, fc, scl * 128:(scl + 1) * 128]
                        nc.tensor.matmul(ps[:, 0:512], h_ap, w2_t[:, fc, 0:512],
                                         start=(fc == 0), stop=(fc == FC - 1))
                        nc.tensor.matmul(ps[:, 512:768], h_ap, w2_t[:, fc, 512:768],
                                         start=(fc == 0), stop=(fc == FC - 1))
                    x1r = ftmp.tile([128, H], F32, tag="x1r")
                    nc.sync.dma_start(out=x1r, in_=x1_scratch[sc * 128:(sc + 1) * 128, :])
                    o2 = ftmp.tile([128, H], F32, tag="o2")
                    nc.vector.tensor_add(o2, ps, b2r_t)
                    nc.vector.tensor_add(o2, o2, x1r)
                    ln_inplace(fstat, o2)
                    nc.sync.dma_start(out=out_d[sc * 128:(sc + 1) * 128, :], in_=o2)

    nc.compile()
    return nc


def _make_runner():
    import jax
    import numpy as _np
    from jax.sharding import Mesh, PartitionSpec, NamedSharding
    from jax.experimental.shard_map import shard_map
    import concourse.mybir as mybir
    from concourse.bass2jax import _bass_exec_p, install_neuronx_cc_hook, partition_id_tensor

    nc = _build_program()
    install_neuronx_cc_hook()

    in_allocs, out_allocs = [], []
    for alloc in nc.m.functions[0].allocations:
        if not isinstance(alloc, mybir.MemoryLocationSet):
            continue
        name = alloc.memorylocations[0].name
        if alloc.kind == "ExternalInput":
            in_allocs.append((name, tuple(alloc.tensor_shape), mybir.dt.np(alloc.dtype)))
        elif alloc.kind == "ExternalOutput":
            out_allocs.append((name, tuple(alloc.tensor_shape), mybir.dt.np(alloc.dtype)))

    part_name = nc.partition_id_tensor.name if nc.partition_id_tensor else None
    in_allocs = [t for t in in_allocs if t[0] != part_name]
    in_names = [n for n, _, _ in in_allocs]
    out_names = [n for n, _, _ in out_allocs]
    out_avals = tuple(jax.core.ShapedArray(s, d) for _, s, d in out_allocs)
    all_names = tuple(in_names + out_names + ([part_name] if part_name else []))

    def _body(*args):
        operands = list(args)
        if part_name:
            operands.append(partition_id_tensor())
        outs = _bass_exec_p.bind(
            *operands,
            out_avals=out_avals,
            in_names=all_names,
            out_names=tuple(out_names),
            lowering_input_output_aliases=(),
            sim_require_finite=True,
            sim_require_nnan=True,
            nc=nc,
        )
        return tuple(outs)

    devices = jax.devices()[:N_CORES]
    mesh = Mesh(_np.asarray(devices), ("core",))
    n_all = len(in_names) + len(out_names)
    sharded = jax.jit(
        shard_map(_body, mesh=mesh, in_specs=(PartitionSpec("core"),) * n_all,
                  out_specs=(PartitionSpec("core"),) * len(out_names), check_rep=False),
        keep_unused=True,
    )
    sh = NamedSharding(mesh, PartitionSpec("core"))
    return {
        "sharded": sharded, "sharding": sh,
        "in_names": in_names, "out_names": out_names, "out_allocs": out_allocs,
    }


def get_runner():
    global _RUNNER
    if _RUNNER is None:
        _RUNNER = _make_runner()
    return _RUNNER


def shard_inputs(inputs):
    """Build the concatenated (over cores, axis 0) input arrays for the SPMD program."""
    bf16 = ml_dtypes.bfloat16
    X = np.asarray(inputs["X"], np.float32)
    WQf = np.ascontiguousarray(np.transpose(np.asarray(inputs["WQ"], np.float32), (1, 0, 2)).reshape(H, H))
    WKf = np.ascontiguousarray(np.transpose(np.asarray(inputs["WK"], np.float32), (1, 0, 2)).reshape(H, H))
    WVf = np.ascontiguousarray(np.transpose(np.asarray(inputs["WV"], np.float32), (1, 0, 2)).reshape(H, H))
    WO = np.asarray(inputs["WO"], np.float32)
    W1 = np.asarray(inputs["W1"], np.float32)
    W2 = np.asarray(inputs["W2"], np.float32)
    b1 = np.asarray(inputs["b1"], np.float32)
    b2 = np.asarray(inputs["b2"], np.float32)
    gamma = np.asarray(inputs["gamma"], np.float32)
    beta = np.asarray(inputs["beta"], np.float32)

    shared = {
        "wq": WQf.astype(bf16), "wk": WKf.astype(bf16), "wv": WVf.astype(bf16),
        "wo": WO.astype(bf16), "w1": W1.astype(bf16), "w2": W2.astype(bf16),
        "b1c": np.ascontiguousarray(b1.reshape(FC, 128).T),
        "b2r": np.broadcast_to(b2, (128, H)).copy(),
        "gr": np.broadcast_to(gamma, (128, H)).copy(),
        "br": np.broadcast_to(beta, (128, H)).copy(),
    }
    per_core = []
    for k in range(N_CORES):
        n_s = 8 + k // 4
        b = k % 4
        qs = X[n_s][256 * b:256 * (b + 1)]
        xq = np.concatenate([X[k], qs], axis=0)          # [1280, 768]
        m = dict(shared)
        m["xt_q"] = np.ascontiguousarray(xq.T).astype(bf16)
        m["xt_s"] = np.ascontiguousarray(X[n_s].T).astype(bf16)
        m["x_res"] = np.ascontiguousarray(xq)
        per_core.append(m)
    return per_core


def assemble_output(results):
    """results: list of 8 per-core 'out' arrays [1280, 768] -> full [10, 1024, 768]."""
    out = np.zeros((N, S, H), np.float32)
    for k in range(N_CORES):
        out[k] = results[k][:1024]
        n_s = 8 + k // 4
        b = k % 4
        out[n_s][256 * b:256 * (b + 1)] = results[k][1024:1280]
    return out


def kernel(**inputs):
    import jax
    r = get_runner()
    per_core = shard_inputs(inputs)
    concat_in = [np.concatenate([per_core[c][name] for c in range(N_CORES)], axis=0)
                 for name in r["in_names"]]
    zeros = [np.zeros((N_CORES * s[0], *s[1:]), d) for _, s, d in r["out_allocs"]]
    dev_in = [jax.device_put(a, r["sharding"]) for a in concat_in + zeros]
    outs = r["sharded"](*dev_in)
    jax.block_until_ready(outs)
    oidx = r["out_names"].index("out")
    o = np.asarray(outs[oidx]).reshape(N_CORES, SQ, H)
    full = assemble_output(list(o))
    mask = np.asarray(inputs["mask"])
    if (mask == 0).any():
        full[mask == 0] = np.nan
    return full


# revision 18
# speedup vs baseline: 1.8012x; 1.8012x over previous
"""Trainium2 Bass kernel for a 1-layer transformer encoder (N=10, S=1024, H=768, A=12, F=3072).

Sharding: 8 cores, SPMD, no collectives. Core k computes:
  - primary:   full batch element n=k           (1024 query rows)
  - secondary: rows [256*b, 256*(b+1)) of n=8+k//4, b=k%4   (256 query rows)
Every core runs an identical program over 1280 query rows; K/V for the
secondary batch element are recomputed per core (small duplicated cost, no
cross-core communication).

Matmuls run in bf16 (fp32 PSUM accumulation); softmax/LayerNorm statistics in fp32.
The softmax normalization is folded into the attn transpose: the PE transposes
exp(scores) with rhs = diag(1/denominator) instead of the identity.
"""
import numpy as np
from contextlib import ExitStack

import ml_dtypes

N, S, H, A, F = 10, 1024, 768, 12, 3072
DH = H // A  # 64
EPS = 1e-5
SQ = 1280          # query rows per core
N_CORES = 8
HC = H // 128      # 6  h-chunks
FC = F // 128      # 24 f-chunks
PAIRS = A // 2     # 6  head pairs

_RUNNER = None


def _build_program(reps=1):
    import concourse.mybir as mybir
    import concourse.tile as tile
    from concourse import bacc
    from concourse.masks import make_identity

    BF = mybir.dt.bfloat16
    F32 = mybir.dt.float32
    AF = mybir.ActivationFunctionType
    OP = mybir.AluOpType

    nc = bacc.Bacc("TRN2", target_bir_lowering=False, debug=False, num_devices=N_CORES)

    d = {}
    def din(name, shape, dt):
        d[name] = nc.dram_tensor(name, shape, dt, kind="ExternalInput").ap()

    # All large tensors ship pre-tiled to the on-chip [128 partitions, ...] layout so
    # every DMA is 128 long contiguous runs.
    din("xt_q", [128, HC * SQ], BF)    # X^T of 1280 query rows, h-chunked
    din("xt_s", [128, HC * S], BF)     # X^T of secondary batch element
    din("x_res", [128, 10 * H], BF)    # X rows (s-chunked) for the first residual
    din("wq", [128, HC * H], BF)
    din("wk", [128, HC * H], BF)
    din("wv", [128, HC * H], BF)
    din("wo", [128, HC * H], BF)
    din("w1", [128, HC * F], BF)
    din("w2", [128, FC * H], BF)
    din("b1c", [128, FC], F32)         # col j = b1[128j:128(j+1)]
    din("g_row", [1, H], F32)
    din("b_row", [1, H], F32)
    din("b2_row", [1, H], F32)
    out_d = nc.dram_tensor("out", [SQ, H], F32, kind="ExternalOutput").ap()

    S_BLOCKS = [(0, 512), (512, 512), (1024, 256)]  # query-dim blocking

    with tile.TileContext(nc) as tc:
      for _rep in range(reps):
       with ExitStack() as ctx:
        glob = ctx.enter_context(tc.tile_pool(name="glob", bufs=1))
        x1t = glob.tile([128, HC, SQ], BF)        # X1^T for FFN1
        ident = glob.tile([128, 128], BF)
        make_identity(nc, ident)
        ones_row = glob.tile([1, 64], F32)
        nc.vector.memset(ones_row, 1.0)
        eps_t = glob.tile([128, 1], F32)
        nc.vector.memset(eps_t, EPS)
        gr_t = glob.tile([128, H], F32)
        nc.sync.dma_start(out=gr_t, in_=d["g_row"].to_broadcast([128, H]))
        br_t = glob.tile([128, H], F32)
        nc.sync.dma_start(out=br_t, in_=d["b_row"].to_broadcast([128, H]))
        b2r_t = glob.tile([128, H], F32)
        nc.sync.dma_start(out=b2r_t, in_=d["b2_row"].to_broadcast([128, H]))
        b1c_t = glob.tile([128, FC], F32)
        nc.sync.dma_start(out=b1c_t, in_=d["b1c"])
        x1 = glob.tile([128, 10, H], F32)      # LN1 output, SBUF-resident
        def ln_inplace(pool, x):
            """LayerNorm x (f32 [128, 768]) in place: (x-mu)*rsqrt(var+eps)*gamma+beta."""
            stats = pool.tile([128, 2, 6], F32)
            nc.vector.bn_stats(stats[:, 0, :], x[:, 0:384])
            nc.vector.bn_stats(stats[:, 1, :], x[:, 384:768])
            mv = pool.tile([128, 2], F32)
            nc.vector.bn_aggr(mv, stats)
            std = pool.tile([128, 1], F32)
            nc.scalar.activation(std, mv[:, 1:2], AF.Sqrt, bias=eps_t)
            rstd = pool.tile([128, 1], F32)
            nc.vector.reciprocal(rstd, std)
            nc.vector.tensor_scalar(x, x, mv[:, 0:1], rstd, OP.subtract, OP.mult)
            nc.vector.tensor_mul(x, x, gr_t)
            nc.vector.tensor_add(x, x, br_t)

        with ExitStack() as zctx:  # zt lives through attention + O-projection
            zpool = zctx.enter_context(tc.tile_pool(name="zpool", bufs=1))
            zt = zpool.tile([128, PAIRS, SQ], BF)     # Z^T

            with ExitStack() as actx:  # ---- projections + attention core ----
                apool = actx.enter_context(tc.tile_pool(name="apool", bufs=1))
                qt = apool.tile([128, PAIRS, SQ], BF)     # Q^T/8, head-pair-major rows
                kt = [apool.tile([128, PAIRS, S], BF, name=f"kt{s}", tag=f"kt{s}")
                      for s in range(2)]
                # V normal, t-chunked, 65 cols per head: col 64 = 1.0 (softmax denom trick)
                vt = [apool.tile([128, 8, A * 65], BF, name=f"vt{s}", tag=f"vt{s}")
                      for s in range(2)]
                for s in range(2):
                    nc.vector.memset(
                        vt[s].rearrange("p t (a e) -> p t a e", e=65)[:, :, :, 64:65], 1.0)

                with ExitStack() as pctx:  # -- projections --
                    ppool = pctx.enter_context(tc.tile_pool(name="ppool", bufs=1))
                    xtq_t = ppool.tile([128, HC, SQ], BF)
                    _xq = d["xt_q"].rearrange("p (c s) -> p c s", s=SQ)
                    nc.sync.dma_start(out=xtq_t[:, 0:3, :], in_=_xq[:, 0:3, :])
                    nc.gpsimd.dma_start(out=xtq_t[:, 3:6, :], in_=_xq[:, 3:6, :])
                    xts_t = ppool.tile([128, HC, S], BF)
                    _xs = d["xt_s"].rearrange("p (c s) -> p c s", s=S)
                    nc.scalar.dma_start(out=xts_t[:, 0:3, :], in_=_xs[:, 0:3, :])
                    nc.gpsimd.dma_start(out=xts_t[:, 3:6, :], in_=_xs[:, 3:6, :])
                    wq_t = ppool.tile([128, HC, H], BF)
                    nc.sync.dma_start(out=wq_t, in_=d["wq"].rearrange("p (c n) -> p c n", n=H))
                    wk_t = ppool.tile([128, HC, H], BF)
                    nc.sync.dma_start(out=wk_t, in_=d["wk"].rearrange("p (c n) -> p c n", n=H))
                    wv_t = ppool.tile([128, HC, H], BF)
                    nc.sync.dma_start(out=wv_t, in_=d["wv"].rearrange("p (c n) -> p c n", n=H))

                    ps512 = pctx.enter_context(tc.tile_pool(name="ps512", bufs=3, space="PSUM"))
                    ps768 = pctx.enter_context(tc.tile_pool(name="ps768", bufs=2, space="PSUM"))

                    # Q^T (scaled by 1/8): out [d-pair 128, s] = wq_pair^T @ X^T
                    for pair in range(PAIRS):
                        for s0, sl in S_BLOCKS:
                            ps = ps512.tile([128, 512], F32)
                            for kc in range(HC):
                                nc.tensor.matmul(
                                    ps[:, :sl],
                                    wq_t[:, kc, pair * 128:(pair + 1) * 128],
                                    xtq_t[:, kc, s0:s0 + sl],
                                    start=(kc == 0), stop=(kc == HC - 1))
                            nc.scalar.activation(qt[:, pair, s0:s0 + sl], ps[:, :sl],
                                                 AF.Copy, scale=0.125)

                    for st in range(2):  # KV sets: 0 = primary, 1 = secondary
                        src = xtq_t if st == 0 else xts_t
                        # K^T
                        for pair in range(PAIRS):
                            for t0 in (0, 512):
                                ps = ps512.tile([128, 512], F32)
                                for kc in range(HC):
                                    nc.tensor.matmul(
                                        ps,
                                        wk_t[:, kc, pair * 128:(pair + 1) * 128],
                                        src[:, kc, t0:t0 + 512],
                                        start=(kc == 0), stop=(kc == HC - 1))
                                nc.vector.tensor_copy(kt[st][:, pair, t0:t0 + 512], ps)
                        # V (normal layout): out [t-chunk 128, a*64+d]
                        for t8 in range(8):
                            ps = ps768.tile([128, H], F32)
                            for kc in range(HC):
                                nc.tensor.matmul(ps[:, 0:512], src[:, kc, t8 * 128:(t8 + 1) * 128],
                                                 wv_t[:, kc, 0:512],
                                                 start=(kc == 0), stop=(kc == HC - 1))
                                nc.tensor.matmul(ps[:, 512:768], src[:, kc, t8 * 128:(t8 + 1) * 128],
                                                 wv_t[:, kc, 512:768],
                                                 start=(kc == 0), stop=(kc == HC - 1))
                            nc.vector.tensor_copy(
                                vt[st][:, t8, :].rearrange("p (a e) -> p a e", e=65)[:, :, 0:64],
                                ps.rearrange("p (a e) -> p a e", e=64))

                # prefetch W1 now: overlaps the whole attention core.
                # (pool entered on the outer stack so it survives until FFN)
                fpool1 = ctx.enter_context(tc.tile_pool(name="fpool1", bufs=1, side="right"))
                w1_t = fpool1.tile([128, HC, F], BF)
                _w1 = d["w1"].rearrange("p (c n) -> p c n", n=F)
                nc.sync.dma_start(out=w1_t[:, 0:3, :], in_=_w1[:, 0:3, :])
                nc.gpsimd.dma_start(out=w1_t[:, 3:6, :], in_=_w1[:, 3:6, :])

                with ExitStack() as attx:  # -- attention core (scoresT, 2 s-chunks per group) --
                    etp = attx.enter_context(tc.tile_pool(name="etp", bufs=3))
                    dens = attx.enter_context(tc.tile_pool(name="dens", bufs=6))
                    ps_st = attx.enter_context(tc.tile_pool(name="ps_st", bufs=1, space="PSUM"))
                    ps_z = attx.enter_context(tc.tile_pool(name="ps_z", bufs=2, space="PSUM"))
                    ps_b = attx.enter_context(tc.tile_pool(name="ps_b", bufs=2, space="PSUM"))

                    for a in range(A):
                        pair, r0 = a // 2, (a % 2) * 64
                        for g0 in (0, 2, 4, 6, 8):
                            st = 0 if g0 < 8 else 1
                            q_ap = qt[r0:r0 + 64, pair, g0 * 128:(g0 + 2) * 128]  # [64, 256]
                            # scoresT[t, s] in two half-tiles so exp pipelines with the PE
                            psA = ps_st.tile([128, 4, 256], F32)
                            psB = ps_st.tile([128, 4, 256], F32, tag="psB")
                            for t8 in range(8):
                                tgt = psA if t8 < 4 else psB
                                nc.tensor.matmul(
                                    tgt[:, t8 % 4, :],
                                    kt[st][r0:r0 + 64, pair, t8 * 128:(t8 + 1) * 128],
                                    q_ap, start=True, stop=True)
                            expT = etp.tile([128, 8, 256], BF)
                            nc.scalar.activation(expT[:, 0:4, :], psA, AF.Exp)
                            nc.scalar.activation(expT[:, 4:8, :], psB, AF.Exp)
                            # Z' = [V | 1]^T-chunks @ expT  (row 64 = softmax denominator)
                            ps_zt = ps_z.tile([65, 256], F32)
                            for t8 in range(8):
                                nc.tensor.matmul(ps_zt, vt[st][:, t8, a * 65:(a + 1) * 65],
                                                 expT[:, t8, :],
                                                 start=(t8 == 0), stop=(t8 == 7))
                            den = dens.tile([1, 256], F32)
                            nc.vector.tensor_copy(den, ps_zt[64:65, :])
                            ps_bc = ps_b.tile([64, 256], F32)
                            nc.tensor.matmul(ps_bc, ones_row, den, start=True, stop=True)
                            recip = dens.tile([64, 256], F32, tag="recip")
                            nc.vector.reciprocal(recip, ps_bc)
                            nc.vector.tensor_mul(
                                zt[r0:r0 + 64, pair, g0 * 128:(g0 + 2) * 128],
                                ps_zt[0:64, :], recip)

            with ExitStack() as octx:  # -- O projection + residual + LN1 --
                opool = octx.enter_context(tc.tile_pool(name="opool", bufs=1))
                wo_t = opool.tile([128, HC, H], BF)
                nc.sync.dma_start(out=wo_t, in_=d["wo"].rearrange("p (c n) -> p c n", n=H))
                # prefetch W2: overlaps O-projection + FFN1
                fpool2 = ctx.enter_context(tc.tile_pool(name="fpool2", bufs=1, side="right"))
                w2_t = fpool2.tile([128, FC, H], BF)
                _w2 = d["w2"].rearrange("p (c n) -> p c n", n=H)
                nc.sync.dma_start(out=w2_t[:, 0:12, :], in_=_w2[:, 0:12, :])
                nc.gpsimd.dma_start(out=w2_t[:, 12:24, :], in_=_w2[:, 12:24, :])
                xres_t = opool.tile([128, 10, H], BF)
                nc.sync.dma_start(out=xres_t, in_=d["x_res"].rearrange("p (s n) -> p s n", n=H))
                otmp = octx.enter_context(tc.tile_pool(name="otmp", bufs=3))
                ostat = octx.enter_context(tc.tile_pool(name="ostat", bufs=6))
                ps_o = octx.enter_context(tc.tile_pool(name="ps_o", bufs=2, space="PSUM"))
                ps_tr = octx.enter_context(tc.tile_pool(name="ps_tr", bufs=2, space="PSUM"))

                for sc in range(10):
                    ps = ps_o.tile([128, H], F32)
                    for kc in range(HC):
                        z_ap = zt[:, kc, sc * 128:(sc + 1) * 128]
                        nc.tensor.matmul(ps[:, 0:512], z_ap, wo_t[:, kc, 0:512],
                                         start=(kc == 0), stop=(kc == HC - 1))
                        nc.tensor.matmul(ps[:, 512:768], z_ap, wo_t[:, kc, 512:768],
                                         start=(kc == 0), stop=(kc == HC - 1))
                    nc.vector.tensor_add(x1[:, sc, :], ps, xres_t[:, sc, :])
                    ln_inplace(ostat, x1[:, sc, :])
                    x1b = otmp.tile([128, H], BF, tag="x1b")
                    nc.scalar.copy(x1b, x1[:, sc, :])
                    for hc2 in range(HC):
                        ps_x = ps_tr.tile([128, 128], F32)
                        nc.tensor.matmul(ps_x, x1b[:, hc2 * 128:(hc2 + 1) * 128], ident,
                                         start=True, stop=True)
                        nc.vector.tensor_copy(x1t[:, hc2, sc * 128:(sc + 1) * 128], ps_x)

        with ExitStack() as fctx:  # ---- FFN phase ----
            hidp = fctx.enter_context(tc.tile_pool(name="hidp", bufs=2))
            ftmp = fctx.enter_context(tc.tile_pool(name="ftmp", bufs=3))
            fstat = fctx.enter_context(tc.tile_pool(name="fstat", bufs=6))
            ps_h = fctx.enter_context(tc.tile_pool(name="ps_h", bufs=3, space="PSUM"))
            ps_f = fctx.enter_context(tc.tile_pool(name="ps_f", bufs=2, space="PSUM"))

            for s0, sl in S_BLOCKS:
                hid = hidp.tile([128, FC, 512], BF)
                for fc in range(FC):
                    ps = ps_h.tile([128, 512], F32)
                    for kc in range(HC):
                        nc.tensor.matmul(ps[:, :sl], w1_t[:, kc, fc * 128:(fc + 1) * 128],
                                         x1t[:, kc, s0:s0 + sl],
                                         start=(kc == 0), stop=(kc == HC - 1))
                    nc.scalar.activation(hid[:, fc, :sl], ps[:, :sl], AF.Gelu,
                                         bias=b1c_t[:, fc:fc + 1])
                for scl in range(sl // 128):
                    sc = s0 // 128 + scl
                    ps = ps_f.tile([128, H], F32)
                    for fc in range(FC):
                        h_ap = hid[:, fc, scl * 128:(scl + 1) * 128]
                        nc.tensor.matmul(ps[:, 0:512], h_ap, w2_t[:, fc, 0:512],
                                         start=(fc == 0), stop=(fc == FC - 1))
                        nc.tensor.matmul(ps[:, 512:768], h_ap, w2_t[:, fc, 512:768],
                                         start=(fc == 0), stop=(fc == FC - 1))
                    o2 = ftmp.tile([128, H], F32, tag="o2")
                    nc.vector.tensor_add(o2, ps, b2r_t)
                    nc.vector.tensor_add(o2, o2, x1[:, sc, :])
                    ln_inplace(fstat, o2)
                    nc.sync.dma_start(out=out_d[sc * 128:(sc + 1) * 128, :], in_=o2)

    nc.compile()
    return nc


def _make_runner(reps=1):
    import jax
    import numpy as _np
    from jax.sharding import Mesh, PartitionSpec, NamedSharding
    from jax.experimental.shard_map import shard_map
    import concourse.mybir as mybir
    from concourse.bass2jax import _bass_exec_p, install_neuronx_cc_hook, partition_id_tensor

    nc = _build_program(reps)
    install_neuronx_cc_hook()

    in_allocs, out_allocs = [], []
    for alloc in nc.m.functions[0].allocations:
        if not isinstance(alloc, mybir.MemoryLocationSet):
            continue
        name = alloc.memorylocations[0].name
        if alloc.kind == "ExternalInput":
            in_allocs.append((name, tuple(alloc.tensor_shape), mybir.dt.np(alloc.dtype)))
        elif alloc.kind == "ExternalOutput":
            out_allocs.append((name, tuple(alloc.tensor_shape), mybir.dt.np(alloc.dtype)))

    part_name = nc.partition_id_tensor.name if nc.partition_id_tensor else None
    in_allocs = [t for t in in_allocs if t[0] != part_name]
    in_names = [n for n, _, _ in in_allocs]
    out_names = [n for n, _, _ in out_allocs]
    out_avals = tuple(jax.core.ShapedArray(s, d) for _, s, d in out_allocs)
    all_names = tuple(in_names + out_names + ([part_name] if part_name else []))

    def _body(*args):
        operands = list(args)
        if part_name:
            operands.append(partition_id_tensor())
        outs = _bass_exec_p.bind(
            *operands,
            out_avals=out_avals,
            in_names=all_names,
            out_names=tuple(out_names),
            lowering_input_output_aliases=(),
            sim_require_finite=True,
            sim_require_nnan=True,
            nc=nc,
        )
        return tuple(outs)

    devices = jax.devices()[:N_CORES]
    mesh = Mesh(_np.asarray(devices), ("core",))
    n_all = len(in_names) + len(out_names)
    sharded = jax.jit(
        shard_map(_body, mesh=mesh, in_specs=(PartitionSpec("core"),) * n_all,
                  out_specs=(PartitionSpec("core"),) * len(out_names), check_rep=False),
        keep_unused=True,
    )
    sh = NamedSharding(mesh, PartitionSpec("core"))
    return {
        "sharded": sharded, "sharding": sh,
        "in_names": in_names, "out_names": out_names, "out_allocs": out_allocs,
    }


def get_runner():
    global _RUNNER
    if _RUNNER is None:
        _RUNNER = _make_runner()
    return _RUNNER


def shard_inputs(inputs):
    """Build the per-core input maps for the SPMD program."""
    bf16 = ml_dtypes.bfloat16
    X = np.asarray(inputs["X"], np.float32)
    WQf = np.ascontiguousarray(np.transpose(np.asarray(inputs["WQ"], np.float32), (1, 0, 2)).reshape(H, H))
    WKf = np.ascontiguousarray(np.transpose(np.asarray(inputs["WK"], np.float32), (1, 0, 2)).reshape(H, H))
    WVf = np.ascontiguousarray(np.transpose(np.asarray(inputs["WV"], np.float32), (1, 0, 2)).reshape(H, H))
    WO = np.asarray(inputs["WO"], np.float32)
    W1 = np.asarray(inputs["W1"], np.float32)
    W2 = np.asarray(inputs["W2"], np.float32)
    b1 = np.asarray(inputs["b1"], np.float32)
    b2 = np.asarray(inputs["b2"], np.float32)
    gamma = np.asarray(inputs["gamma"], np.float32)
    beta = np.asarray(inputs["beta"], np.float32)

    def t128(a):
        """[R, C] -> [128, (R//128)*C] pre-tiled so SBUF partition p holds rows p, 128+p, ..."""
        R, C = a.shape
        return np.ascontiguousarray(a.reshape(R // 128, 128, C).transpose(1, 0, 2).reshape(128, -1))

    shared = {
        "wq": t128(WQf).astype(bf16), "wk": t128(WKf).astype(bf16), "wv": t128(WVf).astype(bf16),
        "wo": t128(WO).astype(bf16), "w1": t128(W1).astype(bf16), "w2": t128(W2).astype(bf16),
        "b1c": np.ascontiguousarray(b1.reshape(FC, 128).T),
        "g_row": gamma[None, :].copy(), "b_row": beta[None, :].copy(),
        "b2_row": b2[None, :].copy(),
    }
    per_core = []
    for k in range(N_CORES):
        n_s = 8 + k // 4
        b = k % 4
        qs = X[n_s][256 * b:256 * (b + 1)]
        xq = np.concatenate([X[k], qs], axis=0)          # [1280, 768]
        m = dict(shared)
        m["xt_q"] = t128(np.ascontiguousarray(xq.T)).astype(bf16)
        m["xt_s"] = t128(np.ascontiguousarray(X[n_s].T)).astype(bf16)
        m["x_res"] = t128(xq).astype(bf16)
        per_core.append(m)
    return per_core


def assemble_output(results):
    """results: list of 8 per-core 'out' arrays [1280, 768] -> full [10, 1024, 768]."""
    out = np.zeros((N, S, H), np.float32)
    for k in range(N_CORES):
        out[k] = results[k][:1024]
        n_s = 8 + k // 4
        b = k % 4
        out[n_s][256 * b:256 * (b + 1)] = results[k][1024:1280]
    return out


def kernel(**inputs):
    import jax
    r = get_runner()
    per_core = shard_inputs(inputs)
    concat_in = [np.concatenate([per_core[c][name] for c in range(N_CORES)], axis=0)
                 for name in r["in_names"]]
    zeros = [np.zeros((N_CORES * s[0], *s[1:]), d) for _, s, d in r["out_allocs"]]
    dev_in = [jax.device_put(a, r["sharding"]) for a in concat_in + zeros]
    outs = r["sharded"](*dev_in)
    jax.block_until_ready(outs)
    oidx = r["out_names"].index("out")
    o = np.asarray(outs[oidx]).reshape(N_CORES, SQ, H)
    full = assemble_output(list(o))
    mask = np.asarray(inputs["mask"])
    if (mask == 0).any():
        full[mask == 0] = np.nan
    return full


# revision 20
# speedup vs baseline: 1.8308x; 1.0165x over previous
"""Trainium2 Bass kernel for a 1-layer transformer encoder (N=10, S=1024, H=768, A=12, F=3072).

Sharding: 8 cores, SPMD, no collectives. Core k computes:
  - primary:   full batch element n=k           (1024 query rows)
  - secondary: rows [256*b, 256*(b+1)) of n=8+k//4, b=k%4   (256 query rows)
Every core runs an identical program over 1280 query rows; K/V for the
secondary batch element are recomputed per core (small duplicated cost, no
cross-core communication).

Matmuls run in bf16 (fp32 PSUM accumulation); softmax/LayerNorm statistics in fp32.
The softmax normalization is folded into the attn transpose: the PE transposes
exp(scores) with rhs = diag(1/denominator) instead of the identity.
"""
import numpy as np
from contextlib import ExitStack

import ml_dtypes

N, S, H, A, F = 10, 1024, 768, 12, 3072
DH = H // A  # 64
EPS = 1e-5
SQ = 1280          # query rows per core
N_CORES = 8
HC = H // 128      # 6  h-chunks
FC = F // 128      # 24 f-chunks
PAIRS = A // 2     # 6  head pairs

_RUNNER = None


def _build_program(reps=1):
    import concourse.mybir as mybir
    import concourse.tile as tile
    from concourse import bacc
    from concourse.masks import make_identity

    BF = mybir.dt.bfloat16
    F32 = mybir.dt.float32
    AF = mybir.ActivationFunctionType
    OP = mybir.AluOpType

    nc = bacc.Bacc("TRN2", target_bir_lowering=False, debug=False, num_devices=N_CORES)

    d = {}
    def din(name, shape, dt):
        d[name] = nc.dram_tensor(name, shape, dt, kind="ExternalInput").ap()

    # All large tensors ship pre-tiled to the on-chip [128 partitions, ...] layout so
    # every DMA is 128 long contiguous runs.
    din("xt_q", [128, HC * SQ], BF)    # X^T of 1280 query rows, h-chunked
    din("xt_s", [128, HC * S], BF)     # X^T of secondary batch element
    din("x_res", [128, 10 * H], BF)    # X rows (s-chunked) for the first residual
    din("wq", [128, HC * H], BF)
    din("wk", [128, HC * H], BF)
    din("wv", [128, HC * H], BF)
    din("wo", [128, HC * H], BF)
    din("w1", [128, HC * F], BF)
    din("w2", [128, FC * H], BF)
    din("b1c", [128, FC], F32)         # col j = b1[128j:128(j+1)]
    din("g_row", [1, H], F32)
    din("b_row", [1, H], F32)
    din("b2_row", [1, H], F32)
    out_d = nc.dram_tensor("out", [SQ, H], F32, kind="ExternalOutput").ap()

    S_BLOCKS = [(0, 512), (512, 512), (1024, 256)]  # query-dim blocking

    with tile.TileContext(nc) as tc:
      for _rep in range(reps):
       with ExitStack() as ctx:
        glob = ctx.enter_context(tc.tile_pool(name="glob", bufs=1))
        x1t = glob.tile([128, HC, SQ], BF)        # X1^T for FFN1
        ident = glob.tile([128, 128], BF)
        make_identity(nc, ident)
        ones_row = glob.tile([1, 64], F32)
        nc.vector.memset(ones_row, 1.0)
        eps_t = glob.tile([128, 1], F32)
        nc.vector.memset(eps_t, EPS)
        gr_t = glob.tile([128, H], F32)
        nc.sync.dma_start(out=gr_t, in_=d["g_row"].to_broadcast([128, H]))
        br_t = glob.tile([128, H], F32)
        nc.sync.dma_start(out=br_t, in_=d["b_row"].to_broadcast([128, H]))
        b2r_t = glob.tile([128, H], F32)
        nc.sync.dma_start(out=b2r_t, in_=d["b2_row"].to_broadcast([128, H]))
        b1c_t = glob.tile([128, FC], F32)
        nc.sync.dma_start(out=b1c_t, in_=d["b1c"])
        x1 = glob.tile([128, 10, H], F32)      # LN1 output, SBUF-resident
        def ln_inplace(pool, x):
            """LayerNorm x (f32 [128, 768]) in place: (x-mu)*rsqrt(var+eps)*gamma+beta."""
            stats = pool.tile([128, 2, 6], F32)
            nc.vector.bn_stats(stats[:, 0, :], x[:, 0:384])
            nc.vector.bn_stats(stats[:, 1, :], x[:, 384:768])
            mv = pool.tile([128, 2], F32)
            nc.vector.bn_aggr(mv, stats)
            std = pool.tile([128, 1], F32)
            nc.scalar.activation(std, mv[:, 1:2], AF.Sqrt, bias=eps_t)
            rstd = pool.tile([128, 1], F32)
            nc.vector.reciprocal(rstd, std)
            nc.vector.tensor_scalar(x, x, mv[:, 0:1], rstd, OP.subtract, OP.mult)
            nc.vector.tensor_mul(x, x, gr_t)
            nc.vector.tensor_add(x, x, br_t)

        with ExitStack() as zctx:  # zt lives through attention + O-projection
            zpool = zctx.enter_context(tc.tile_pool(name="zpool", bufs=1))
            zt = zpool.tile([128, PAIRS, SQ], BF)     # Z^T

            with ExitStack() as actx:  # ---- projections + attention core ----
                apool = actx.enter_context(tc.tile_pool(name="apool", bufs=1))
                qt = apool.tile([128, PAIRS, SQ], BF)     # Q^T/8, head-pair-major rows
                kt = [apool.tile([128, PAIRS, S], BF, name=f"kt{s}", tag=f"kt{s}")
                      for s in range(2)]
                # V normal, t-chunked, 65 cols per head: col 64 = 1.0 (softmax denom trick)
                vt = [apool.tile([128, 8, A * 65], BF, name=f"vt{s}", tag=f"vt{s}")
                      for s in range(2)]
                for s in range(2):
                    nc.vector.memset(
                        vt[s].rearrange("p t (a e) -> p t a e", e=65)[:, :, :, 64:65], 1.0)

                with ExitStack() as pctx:  # -- projections --
                    ppool = pctx.enter_context(tc.tile_pool(name="ppool", bufs=1))
                    xtq_t = ppool.tile([128, HC, SQ], BF)
                    _xq = d["xt_q"].rearrange("p (c s) -> p c s", s=SQ)
                    nc.sync.dma_start(out=xtq_t[:, 0:3, :], in_=_xq[:, 0:3, :])
                    nc.gpsimd.dma_start(out=xtq_t[:, 3:6, :], in_=_xq[:, 3:6, :])
                    xts_t = ppool.tile([128, HC, S], BF)
                    _xs = d["xt_s"].rearrange("p (c s) -> p c s", s=S)
                    nc.scalar.dma_start(out=xts_t[:, 0:3, :], in_=_xs[:, 0:3, :])
                    nc.gpsimd.dma_start(out=xts_t[:, 3:6, :], in_=_xs[:, 3:6, :])
                    wq_t = ppool.tile([128, HC, H], BF)
                    nc.sync.dma_start(out=wq_t, in_=d["wq"].rearrange("p (c n) -> p c n", n=H))
                    wk_t = ppool.tile([128, HC, H], BF)
                    nc.sync.dma_start(out=wk_t, in_=d["wk"].rearrange("p (c n) -> p c n", n=H))
                    wv_t = ppool.tile([128, HC, H], BF)
                    nc.sync.dma_start(out=wv_t, in_=d["wv"].rearrange("p (c n) -> p c n", n=H))

                    ps512 = pctx.enter_context(tc.tile_pool(name="ps512", bufs=3, space="PSUM"))
                    ps768 = pctx.enter_context(tc.tile_pool(name="ps768", bufs=2, space="PSUM"))

                    # Q^T (scaled by 1/8): out [d-pair 128, s] = wq_pair^T @ X^T
                    for pair in range(PAIRS):
                        for s0, sl in S_BLOCKS:
                            ps = ps512.tile([128, 512], F32)
                            for kc in range(HC):
                                nc.tensor.matmul(
                                    ps[:, :sl],
                                    wq_t[:, kc, pair * 128:(pair + 1) * 128],
                                    xtq_t[:, kc, s0:s0 + sl],
                                    start=(kc == 0), stop=(kc == HC - 1))
                            nc.scalar.activation(qt[:, pair, s0:s0 + sl], ps[:, :sl],
                                                 AF.Copy, scale=0.125)

                    for st in range(2):  # KV sets: 0 = primary, 1 = secondary
                        src = xtq_t if st == 0 else xts_t
                        # K^T
                        for pair in range(PAIRS):
                            for t0 in (0, 512):
                                ps = ps512.tile([128, 512], F32)
                                for kc in range(HC):
                                    nc.tensor.matmul(
                                        ps,
                                        wk_t[:, kc, pair * 128:(pair + 1) * 128],
                                        src[:, kc, t0:t0 + 512],
                                        start=(kc == 0), stop=(kc == HC - 1))
                                nc.vector.tensor_copy(kt[st][:, pair, t0:t0 + 512], ps)
                        # V (normal layout): out [t-chunk 128, a*64+d]
                        for t8 in range(8):
                            ps = ps768.tile([128, H], F32)
                            for kc in range(HC):
                                nc.tensor.matmul(ps[:, 0:512], src[:, kc, t8 * 128:(t8 + 1) * 128],
                                                 wv_t[:, kc, 0:512],
                                                 start=(kc == 0), stop=(kc == HC - 1))
                                nc.tensor.matmul(ps[:, 512:768], src[:, kc, t8 * 128:(t8 + 1) * 128],
                                                 wv_t[:, kc, 512:768],
                                                 start=(kc == 0), stop=(kc == HC - 1))
                            nc.vector.tensor_copy(
                                vt[st][:, t8, :].rearrange("p (a e) -> p a e", e=65)[:, :, 0:64],
                                ps.rearrange("p (a e) -> p a e", e=64))

                # prefetch W1 now: overlaps the whole attention core.
                # (pool entered on the outer stack so it survives until FFN)
                fpool1 = ctx.enter_context(tc.tile_pool(name="fpool1", bufs=1, side="right"))
                w1_t = fpool1.tile([128, HC, F], BF)
                _w1 = d["w1"].rearrange("p (c n) -> p c n", n=F)
                nc.sync.dma_start(out=w1_t[:, 0:3, :], in_=_w1[:, 0:3, :])
                nc.gpsimd.dma_start(out=w1_t[:, 3:6, :], in_=_w1[:, 3:6, :])

                with ExitStack() as attx:  # -- attention core (scoresT, 2 s-chunks per group) --
                    etp = attx.enter_context(tc.tile_pool(name="etp", bufs=4))
                    dens = attx.enter_context(tc.tile_pool(name="dens", bufs=6))
                    ps_st = attx.enter_context(tc.tile_pool(name="ps_st", bufs=1, space="PSUM"))
                    ps_z = attx.enter_context(tc.tile_pool(name="ps_z", bufs=2, space="PSUM"))
                    ps_b = attx.enter_context(tc.tile_pool(name="ps_b", bufs=2, space="PSUM"))

                    for a in range(A):
                        pair, r0 = a // 2, (a % 2) * 64
                        for g0 in (0, 2, 4, 6, 8):
                            st = 0 if g0 < 8 else 1
                            q_ap = qt[r0:r0 + 64, pair, g0 * 128:(g0 + 2) * 128]  # [64, 256]
                            # scoresT[t, s] in two half-tiles so exp pipelines with the PE
                            psA = ps_st.tile([128, 4, 256], F32)
                            psB = ps_st.tile([128, 4, 256], F32, tag="psB")
                            for t8 in range(8):
                                tgt = psA if t8 < 4 else psB
                                nc.tensor.matmul(
                                    tgt[:, t8 % 4, :],
                                    kt[st][r0:r0 + 64, pair, t8 * 128:(t8 + 1) * 128],
                                    q_ap, start=True, stop=True)
                            expT = etp.tile([128, 8, 256], BF)
                            nc.scalar.activation(expT[:, 0:4, :], psA, AF.Exp)
                            nc.scalar.activation(expT[:, 4:8, :], psB, AF.Exp)
                            # Z' = [V | 1]^T-chunks @ expT  (row 64 = softmax denominator)
                            ps_zt = ps_z.tile([65, 256], F32)
                            for t8 in range(8):
                                nc.tensor.matmul(ps_zt, vt[st][:, t8, a * 65:(a + 1) * 65],
                                                 expT[:, t8, :],
                                                 start=(t8 == 0), stop=(t8 == 7))
                            den = dens.tile([1, 256], F32)
                            nc.vector.tensor_copy(den, ps_zt[64:65, :])
                            ps_bc = ps_b.tile([64, 256], F32)
                            nc.tensor.matmul(ps_bc, ones_row, den, start=True, stop=True)
                            recip = dens.tile([64, 256], F32, tag="recip")
                            nc.vector.reciprocal(recip, ps_bc)
                            nc.vector.tensor_mul(
                                zt[r0:r0 + 64, pair, g0 * 128:(g0 + 2) * 128],
                                ps_zt[0:64, :], recip)

            with ExitStack() as octx:  # -- O projection + residual + LN1 --
                opool = octx.enter_context(tc.tile_pool(name="opool", bufs=1))
                wo_t = opool.tile([128, HC, H], BF)
                nc.sync.dma_start(out=wo_t, in_=d["wo"].rearrange("p (c n) -> p c n", n=H))
                # prefetch W2: overlaps O-projection + FFN1
                fpool2 = ctx.enter_context(tc.tile_pool(name="fpool2", bufs=1, side="right"))
                w2_t = fpool2.tile([128, FC, H], BF)
                _w2 = d["w2"].rearrange("p (c n) -> p c n", n=H)
                nc.sync.dma_start(out=w2_t[:, 0:12, :], in_=_w2[:, 0:12, :])
                nc.gpsimd.dma_start(out=w2_t[:, 12:24, :], in_=_w2[:, 12:24, :])
                xres_t = opool.tile([128, 10, H], BF)
                nc.sync.dma_start(out=xres_t, in_=d["x_res"].rearrange("p (s n) -> p s n", n=H))
                otmp = octx.enter_context(tc.tile_pool(name="otmp", bufs=3))
                ostat = octx.enter_context(tc.tile_pool(name="ostat", bufs=6))
                ps_o = octx.enter_context(tc.tile_pool(name="ps_o", bufs=2, space="PSUM"))
                ps_tr = octx.enter_context(tc.tile_pool(name="ps_tr", bufs=2, space="PSUM"))

                for sc in range(10):
                    ps = ps_o.tile([128, H], F32)
                    for kc in range(HC):
                        z_ap = zt[:, kc, sc * 128:(sc + 1) * 128]
                        nc.tensor.matmul(ps[:, 0:512], z_ap, wo_t[:, kc, 0:512],
                                         start=(kc == 0), stop=(kc == HC - 1))
                        nc.tensor.matmul(ps[:, 512:768], z_ap, wo_t[:, kc, 512:768],
                                         start=(kc == 0), stop=(kc == HC - 1))
                    nc.vector.tensor_add(x1[:, sc, :], ps, xres_t[:, sc, :])
                    ln_inplace(ostat, x1[:, sc, :])
                    x1b = otmp.tile([128, H], BF, tag="x1b")
                    nc.scalar.copy(x1b, x1[:, sc, :])
                    for hc2 in range(HC):
                        ps_x = ps_tr.tile([128, 128], F32)
                        nc.tensor.matmul(ps_x, x1b[:, hc2 * 128:(hc2 + 1) * 128], ident,
                                         start=True, stop=True)
                        nc.vector.tensor_copy(x1t[:, hc2, sc * 128:(sc + 1) * 128], ps_x)

        with ExitStack() as fctx:  # ---- FFN phase ----
            hidp = fctx.enter_context(tc.tile_pool(name="hidp", bufs=2))
            ftmp = fctx.enter_context(tc.tile_pool(name="ftmp", bufs=3))
            fstat = fctx.enter_context(tc.tile_pool(name="fstat", bufs=6))
            ps_h = fctx.enter_context(tc.tile_pool(name="ps_h", bufs=4, space="PSUM"))
            ps_f = fctx.enter_context(tc.tile_pool(name="ps_f", bufs=2, space="PSUM"))

            for s0, sl in S_BLOCKS:
                hid = hidp.tile([128, FC, 512], BF)
                for fc in range(FC):
                    ps = ps_h.tile([128, 512], F32)
                    for kc in range(HC):
                        nc.tensor.matmul(ps[:, :sl], w1_t[:, kc, fc * 128:(fc + 1) * 128],
                                         x1t[:, kc, s0:s0 + sl],
                                         start=(kc == 0), stop=(kc == HC - 1))
                    nc.scalar.activation(hid[:, fc, :sl], ps[:, :sl], AF.Gelu,
                                         bias=b1c_t[:, fc:fc + 1])
                for scl in range(sl // 128):
                    sc = s0 // 128 + scl
                    ps = ps_f.tile([128, H], F32)
                    for fc in range(FC):
                        h_ap = hid[:, fc, scl * 128:(scl + 1) * 128]
                        nc.tensor.matmul(ps[:, 0:512], h_ap, w2_t[:, fc, 0:512],
                                         start=(fc == 0), stop=(fc == FC - 1))
                        nc.tensor.matmul(ps[:, 512:768], h_ap, w2_t[:, fc, 512:768],
                                         start=(fc == 0), stop=(fc == FC - 1))
                    o2 = ftmp.tile([128, H], F32, tag="o2")
                    nc.vector.tensor_add(o2, ps, b2r_t)
                    nc.vector.tensor_add(o2, o2, x1[:, sc, :])
                    ln_inplace(fstat, o2)
                    nc.gpsimd.dma_start(out=out_d[sc * 128:(sc + 1) * 128, :], in_=o2)

    nc.compile()
    return nc


def _make_runner(reps=1):
    import jax
    import numpy as _np
    from jax.sharding import Mesh, PartitionSpec, NamedSharding
    from jax.experimental.shard_map import shard_map
    import concourse.mybir as mybir
    from concourse.bass2jax import _bass_exec_p, install_neuronx_cc_hook, partition_id_tensor

    nc = _build_program(reps)
    install_neuronx_cc_hook()

    in_allocs, out_allocs = [], []
    for alloc in nc.m.functions[0].allocations:
        if not isinstance(alloc, mybir.MemoryLocationSet):
            continue
        name = alloc.memorylocations[0].name
        if alloc.kind == "ExternalInput":
            in_allocs.append((name, tuple(alloc.tensor_shape), mybir.dt.np(alloc.dtype)))
        elif alloc.kind == "ExternalOutput":
            out_allocs.append((name, tuple(alloc.tensor_shape), mybir.dt.np(alloc.dtype)))

    part_name = nc.partition_id_tensor.name if nc.partition_id_tensor else None
    in_allocs = [t for t in in_allocs if t[0] != part_name]
    in_names = [n for n, _, _ in in_allocs]
    out_names = [n for n, _, _ in out_allocs]
    out_avals = tuple(jax.core.ShapedArray(s, d) for _, s, d in out_allocs)
    all_names = tuple(in_names + out_names + ([part_name] if part_name else []))

    def _body(*args):
        operands = list(args)
        if part_name:
            operands.append(partition_id_tensor())
        outs = _bass_exec_p.bind(
            *operands,
            out_avals=out_avals,
            in_names=all_names,
            out_names=tuple(out_names),
            lowering_input_output_aliases=(),
            sim_require_finite=True,
            sim_require_nnan=True,
            nc=nc,
        )
        return tuple(outs)

    devices = jax.devices()[:N_CORES]
    mesh = Mesh(_np.asarray(devices), ("core",))
    n_all = len(in_names) + len(out_names)
    sharded = jax.jit(
        shard_map(_body, mesh=mesh, in_specs=(PartitionSpec("core"),) * n_all,
                  out_specs=(PartitionSpec("core"),) * len(out_names), check_rep=False),
        keep_unused=True,
    )
    sh = NamedSharding(mesh, PartitionSpec("core"))
    return {
        "sharded": sharded, "sharding": sh,
        "in_names": in_names, "out_names": out_names, "out_allocs": out_allocs,
    }


def get_runner():
    global _RUNNER
    if _RUNNER is None:
        _RUNNER = _make_runner()
    return _RUNNER


def shard_inputs(inputs):
    """Build the per-core input maps for the SPMD program."""
    bf16 = ml_dtypes.bfloat16
    X = np.asarray(inputs["X"], np.float32)
    WQf = np.ascontiguousarray(np.transpose(np.asarray(inputs["WQ"], np.float32), (1, 0, 2)).reshape(H, H))
    WKf = np.ascontiguousarray(np.transpose(np.asarray(inputs["WK"], np.float32), (1, 0, 2)).reshape(H, H))
    WVf = np.ascontiguousarray(np.transpose(np.asarray(inputs["WV"], np.float32), (1, 0, 2)).reshape(H, H))
    WO = np.asarray(inputs["WO"], np.float32)
    W1 = np.asarray(inputs["W1"], np.float32)
    W2 = np.asarray(inputs["W2"], np.float32)
    b1 = np.asarray(inputs["b1"], np.float32)
    b2 = np.asarray(inputs["b2"], np.float32)
    gamma = np.asarray(inputs["gamma"], np.float32)
    beta = np.asarray(inputs["beta"], np.float32)

    def t128(a):
        """[R, C] -> [128, (R//128)*C] pre-tiled so SBUF partition p holds rows p, 128+p, ..."""
        R, C = a.shape
        return np.ascontiguousarray(a.reshape(R // 128, 128, C).transpose(1, 0, 2).reshape(128, -1))

    shared = {
        "wq": t128(WQf).astype(bf16), "wk": t128(WKf).astype(bf16), "wv": t128(WVf).astype(bf16),
        "wo": t128(WO).astype(bf16), "w1": t128(W1).astype(bf16), "w2": t128(W2).astype(bf16),
        "b1c": np.ascontiguousarray(b1.reshape(FC, 128).T),
        "g_row": gamma[None, :].copy(), "b_row": beta[None, :].copy(),
        "b2_row": b2[None, :].copy(),
    }
    per_core = []
    for k in range(N_CORES):
        n_s = 8 + k // 4
        b = k % 4
        qs = X[n_s][256 * b:256 * (b + 1)]
        xq = np.concatenate([X[k], qs], axis=0)          # [1280, 768]
        m = dict(shared)
        m["xt_q"] = t128(np.ascontiguousarray(xq.T)).astype(bf16)
        m["xt_s"] = t128(np.ascontiguousarray(X[n_s].T)).astype(bf16)
        m["x_res"] = t128(xq).astype(bf16)
        per_core.append(m)
    return per_core


def assemble_output(results):
    """results: list of 8 per-core 'out' arrays [1280, 768] -> full [10, 1024, 768]."""
    out = np.zeros((N, S, H), np.float32)
    for k in range(N_CORES):
        out[k] = results[k][:1024]
        n_s = 8 + k // 4
        b = k % 4
        out[n_s][256 * b:256 * (b + 1)] = results[k][1024:1280]
    return out


def kernel(**inputs):
    import jax
    r = get_runner()
    per_core = shard_inputs(inputs)
    concat_in = [np.concatenate([per_core[c][name] for c in range(N_CORES)], axis=0)
                 for name in r["in_names"]]
    zeros = [np.zeros((N_CORES * s[0], *s[1:]), d) for _, s, d in r["out_allocs"]]
    dev_in = [jax.device_put(a, r["sharding"]) for a in concat_in + zeros]
    outs = r["sharded"](*dev_in)
    jax.block_until_ready(outs)
    oidx = r["out_names"].index("out")
    o = np.asarray(outs[oidx]).reshape(N_CORES, SQ, H)
    full = assemble_output(list(o))
    mask = np.asarray(inputs["mask"])
    if (mask == 0).any():
        full[mask == 0] = np.nan
    return full
